# revision 1
# baseline (speedup 1.0000x reference)
"""Trainium2 Bass kernel for nn_LocalAttention_28518582845970.

The reference projects the full 256x256x1024 grid through Q/K/V/O but
returns only out[px, py] -- a single 1024-vector.  That vector depends
on one window row: 129 tokens, one query token, and the four 1024x1024
weights (by linearity, softmax shift-invariance, and sum(attn)==1):

    q      = Wq t_q + bq
    u      = Wk^T q                      (the q.bk term is constant in k
                                          -> dropped: softmax invariant)
    scores = tokens @ u
    attn   = softmax(scores/32)
    t_avg  = attn @ tokens
    out_c  = Wo_c (Wv t_avg + bv) + bo_c

v3: zero collectives (measured 25-55us each on this mesh -- they
dominate everything); every core redundantly runs the chain above and
computes only its 128-row slice of the output projection; host
concatenates.  The u-substitution means K and V are never materialized:
the whole kernel is ~100 matmuls of matvec shape.  Matmul operands are
fp16 (fp32 PE matmul is 2-pass/quarter-rate; fp16 is full rate and
halves the 12.6 MiB weight DMA), accumulation is fp32 in PSUM, softmax
and the output projection are fp32.
"""

import os
import sys

os.environ.setdefault("JAX_PLATFORMS", "axon,cpu")

for _p in ("/opt/trn_rl_repo", "/root/.axon_site/_ro/trn_rl_repo"):
    if os.path.isdir(_p) and _p not in sys.path:
        sys.path.append(_p)

import numpy as np

import concourse.bass as bass
import concourse.mybir as mybir
import concourse.tile as tile
from concourse import bacc
from concourse.bass_utils import run_bass_kernel_spmd
from concourse.masks import make_identity

N_CORES = 8
E = 1024
EC = E // 128
WIN = 64
H = W = 256
SCALE = 1.0 / 32.0
F32 = mybir.dt.float32
F16 = mybir.dt.float16

_BUILD_CACHE: dict = {}

# Lighter Tile finale: the stock _drain_and_barrier emits drain + full
# EVSEM barrier + sem clears + second barrier (~10-16us measured on this
# part).  With no collectives and per-core-independent work we keep the
# drain (output DMA completion) and sem clears behind a sem-only
# barrier, dropping the heavy drain-barrier sandwich.
from concourse.vector_clock import ScopedClock as _ScopedClock


def _light_drain_and_barrier(self, tick_clock, wait_clock):
    drain_inst = self.nc.sync.drain()
    wait_clock.add_sem_waits(
        drain_inst.ins, _ScopedClock({None: tick_clock.global_clock})
    )
    self.nc.all_engine_barrier(sem_only=True)
    popped = self.nc._tile_sem_poison_stack.pop()
    assert popped is self._sem_poison
    self.nc.clear_and_free_semaphores(list(self.sems.allocated().values()))
    self.nc.all_engine_barrier(sem_only=True)


tile.TileContext._drain_and_barrier = _light_drain_and_barrier


def _build(L: int, qidx: int):
    KA = min(128, L)          # k-chunk A: tokens [0:KA]
    BS = max(0, L - KA)       # k-chunk B start: tokens [BS:L] (overlap OK)
    nb = 3 * EC + 1           # bias columns: bq(8) bv(8) bo(1) -- packed [128, 17]

    nc = bacc.Bacc(None, target_bir_lowering=False, debug=False)

    tokT_d = nc.dram_tensor("tokT", [E, L], F16, kind="ExternalInput")
    tokN_d = nc.dram_tensor("tokN", [L, E], F16, kind="ExternalInput")
    wqT_d = nc.dram_tensor("wqT", [E, E], F16, kind="ExternalInput")   # (e, f)
    wkN_d = nc.dram_tensor("wkN", [E, E], F16, kind="ExternalInput")   # (f, e) native
    wvT_d = nc.dram_tensor("wvT", [E, E], F16, kind="ExternalInput")   # (e, f)
    woT_d = nc.dram_tensor("woT", [E, 128], F16, kind="ExternalInput")
    bias_d = nc.dram_tensor("biases", [128, 2 * EC + 1], F32, kind="ExternalInput")
    out_d = nc.dram_tensor("out", [128], F32, kind="ExternalOutput")

    wqT_r = wqT_d.rearrange("(c p) f -> p c f", p=128)
    wkN_r = wkN_d.rearrange("(c p) e -> p c e", p=128)
    wvT_r = wvT_d.rearrange("(c p) f -> p c f", p=128)
    FH = [slice(0, 512), slice(512, 1024)]

    with tile.TileContext(nc) as tc:
        with (
            tc.tile_pool(name="consts", bufs=1) as consts,
            tc.tile_pool(name="sbw", bufs=1) as sbw,
            tc.tile_pool(name="psS", bufs=2, space="PSUM") as psS,
        ):
            # ---- loads ----
            tok_sb = consts.tile([128, EC, L], F16)
            nc.sync.dma_start(out=tok_sb, in_=tokT_d.rearrange("(c p) k -> p c k", p=128))
            bias_sb = consts.tile([128, 2 * EC + 1], F32)
            nc.sync.dma_start(out=bias_sb, in_=bias_d[:, :])

            wq_sb = consts.tile([128, EC, E], F16)
            for c in range(EC):
                nc.sync.dma_start(out=wq_sb[:, c, :], in_=wqT_r[:, c, :])
            wk_sb = consts.tile([128, EC, E], F16)
            for c in range(EC):
                nc.sync.dma_start(out=wk_sb[:, c, :], in_=wkN_r[:, c, :])
            wv_sb = consts.tile([128, EC, E], F16)
            for c in range(EC):
                nc.sync.dma_start(out=wv_sb[:, c, :], in_=wvT_r[:, c, :])
            wo_sb = consts.tile([128, EC, 128], F16)
            nc.sync.dma_start(out=wo_sb, in_=woT_d.rearrange("(c p) f -> p c f", p=128))

            tokN_sb = consts.tile([128, EC, 128], F16)
            nc.sync.dma_start(
                out=tokN_sb,
                in_=tokN_d[0:KA].rearrange("k (c p) -> k c p", p=128),
            )
            if L > KA:
                tokNt_sb = consts.tile([L - KA, EC, 128], F16)
                nc.sync.dma_start(
                    out=tokNt_sb,
                    in_=tokN_d[KA:L].rearrange("k (c p) -> k c p", p=128),
                )

            ones16 = consts.tile([1, 128], F16)
            nc.vector.memset(ones16, 1.0)
            warm16 = consts.tile([128, 128], F16)
            nc.vector.memset(warm16, 0.0)

            # PE-HAM warmup: sustained dummy matmuls while weights stream in,
            # so the real chain runs at the unthrottled clock.
            wu_ps = psS.tile([128, 1], F32, tag="wu", bufs=1)
            for w in range(100):
                nc.tensor.matmul(wu_ps, warm16, warm16[:, 0:1],
                                 start=(w == 0), stop=(w == 99))

            # ---- q columns: q[fc] = sum_ec WqT[ec,fc]^T @ t_q (+bq) ----
            # weights stationary ([128,128] fp16 -> fast weight load)
            q_ps = psS.tile([128, EC], F32, tag="qc", bufs=1)
            for fc in range(EC):
                fsl = slice(128 * fc, 128 * (fc + 1))
                for c in range(EC):
                    nc.tensor.matmul(
                        q_ps[:, fc:fc + 1], wq_sb[:, c, fsl],
                        tok_sb[:, c, qidx:qidx + 1],
                        start=(c == 0), stop=(c == EC - 1),
                    )
            q_cols = sbw.tile([128, EC], F16)
            nc.vector.tensor_add(q_cols, q_ps, bias_sb[:, 0:EC])

            # ---- u columns: u[ec] = sum_fc WkN[fc,ec]^T @ q_col[fc] ----
            u_ps = psS.tile([128, EC], F32, tag="uc", bufs=1)
            for ec in range(EC):
                esl = slice(128 * ec, 128 * (ec + 1))
                for c in range(EC):
                    nc.tensor.matmul(
                        u_ps[:, ec:ec + 1], wk_sb[:, c, esl], q_cols[:, c:c + 1],
                        start=(c == 0), stop=(c == EC - 1),
                    )
            u_cols = sbw.tile([128, EC], F16)
            # fold the 1/sqrt(E) score scale into u
            nc.vector.tensor_scalar_mul(u_cols, u_ps, SCALE)

            # ---- scores = u^T @ tokens -> [1, L] directly in row form ----
            s_ps = psS.tile([1, L], F32, tag="sacc", bufs=1)
            for c in range(EC):
                nc.tensor.matmul(s_ps, u_cols[:, c:c + 1], tok_sb[:, c, :],
                                 start=(c == 0), stop=(c == EC - 1))

            wu2_ps = psS.tile([128, 1], F32, tag="wu", bufs=1, name="wu2_ps")
            for w in range(40):
                nc.tensor.matmul(wu2_ps, warm16, warm16[:, 0:1],
                                 start=(w == 0), stop=(w == 39))

            # ---- softmax (scores pre-scaled; |s| <= ~10 so no max-sub
            # needed for fp32 exp -- same result as the reference's
            # max-subtracted softmax) ----
            ex_row = sbw.tile([1, L], F32)
            sm = sbw.tile([1, 1], F32)
            nc.scalar.activation(ex_row, s_ps, mybir.ActivationFunctionType.Exp,
                                 bias=0.0, scale=1.0, accum_out=sm)
            rs = sbw.tile([1, 1], F32)
            nc.vector.reciprocal(rs, sm)
            at16 = sbw.tile([1, L], F16)
            nc.vector.tensor_scalar_mul(at16, ex_row, rs)

            # ---- t_avg = attn @ tokens on PE (tokens in [k, e] layout) ----
            atc_ps = psS.tile([128, 1], F16, tag="s")
            nc.tensor.transpose(atc_ps, at16[0:1, 0:KA], ones16[0:1, 0:1])
            at_colA = sbw.tile([KA, 1], F16)
            nc.vector.tensor_copy(at_colA, atc_ps)
            if L > KA:
                at_tail = sbw.tile([L - KA, 1], F16)
                nc.vector.tensor_copy(at_tail, at16[0:1, KA:L])
            tv_ps = psS.tile([128, EC], F32, tag="tv", bufs=1)
            for c in range(EC):
                nc.tensor.matmul(
                    tv_ps[:, c:c + 1], tokN_sb[:, c, :], at_colA,
                    start=True, stop=(L <= KA),
                )
                if L > KA:
                    nc.tensor.matmul(
                        tv_ps[:, c:c + 1], tokNt_sb[0:1, c, :], at_tail,
                        start=False, stop=True,
                    )
            tv_cols = sbw.tile([128, EC], F16)
            nc.vector.tensor_copy(tv_cols, tv_ps)

            # ---- ctx columns: ctx[fc] = sum_ec WvT[ec,fc]^T @ t_avg[ec] + bv ----
            c_ps = psS.tile([128, EC], F32, tag="cc", bufs=1)
            for fc in range(EC):
                fsl = slice(128 * fc, 128 * (fc + 1))
                for c in range(EC):
                    nc.tensor.matmul(
                        c_ps[:, fc:fc + 1], wv_sb[:, c, fsl], tv_cols[:, c:c + 1],
                        start=(c == 0), stop=(c == EC - 1),
                    )
            ctx_cols = sbw.tile([128, EC], F16)
            nc.vector.tensor_add(ctx_cols, c_ps, bias_sb[:, EC:2 * EC])

            # ---- out_c = WoT_c^T @ ctx + bo_c ----
            o_ps = psS.tile([128, 1], F32, tag="s")
            for c in range(EC):
                nc.tensor.matmul(
                    o_ps, wo_sb[:, c, :], ctx_cols[:, c:c + 1],
                    start=(c == 0), stop=(c == EC - 1),
                )
            o_sb = sbw.tile([128, 1], F32)
            nc.vector.tensor_scalar_add(o_sb, o_ps, bias_sb[:, 2 * EC:2 * EC + 1])
            nc.sync.dma_start(out=out_d.rearrange("(p o) -> p o", o=1), in_=o_sb)

    nc.finalize()
    return nc


def _get_nc(L: int, qidx: int):
    key = (L, qidx)
    if key not in _BUILD_CACHE:
        _BUILD_CACHE[key] = _build(L, qidx)
    return _BUILD_CACHE[key]


def _prep_in_maps(matrix, Wq, bq, Wk, bk, Wv, bv, Wo, bo, px, py):
    px = int(px)
    py = int(py)
    rows = np.arange(H)[px - WIN:px + WIN + 1]
    cols = np.arange(W)[py - WIN:py + WIN + 1]
    L = len(cols)
    gr = rows[px]
    qidx = py

    tokens = np.asarray(matrix[gr][cols], dtype=np.float32)        # [L, E]
    tokT = np.ascontiguousarray(tokens.T).astype(np.float16)       # [E, L]
    tokN = np.ascontiguousarray(tokens).astype(np.float16)         # [L, E]
    wqT = np.ascontiguousarray(np.asarray(Wq, np.float32).T).astype(np.float16)
    wkN = np.ascontiguousarray(np.asarray(Wk, np.float32)).astype(np.float16)
    wvT = np.ascontiguousarray(np.asarray(Wv, np.float32).T).astype(np.float16)

    bq_c = np.asarray(bq, np.float32).reshape(EC, 128).T           # [128, EC]
    bv_c = np.asarray(bv, np.float32).reshape(EC, 128).T

    in_maps = []
    for c in range(N_CORES):
        fc = slice(128 * c, 128 * (c + 1))
        biases = np.concatenate(
            [bq_c, bv_c, np.asarray(bo[fc], np.float32)[:, None]], axis=1
        )
        in_maps.append({
            "tokT": tokT,
            "tokN": tokN,
            "wqT": wqT,
            "wkN": wkN,
            "wvT": wvT,
            "woT": np.ascontiguousarray(np.asarray(Wo, np.float32)[fc].T).astype(np.float16),
            "biases": np.ascontiguousarray(biases),
        })
    return in_maps, L, qidx


def kernel(matrix, Wq, bq, Wk, bk, Wv, bv, Wo, bo, px, py, _trace=False, **_kw):
    in_maps, L, qidx = _prep_in_maps(
        matrix, Wq, bq, Wk, bk, Wv, bv, Wo, bo, px, py
    )
    nc = _get_nc(L, qidx)
    res = run_bass_kernel_spmd(
        nc, in_maps, core_ids=list(range(N_CORES)), trace=_trace
    )
    out = np.concatenate([res.results[c]["out"] for c in range(N_CORES)])
    if _trace:
        return out.astype(np.float32), res
    return out.astype(np.float32)



# revision 2
# speedup vs baseline: 1.2756x; 1.2756x over previous
"""Trainium2 Bass kernel for nn_LocalAttention_28518582845970.

The reference projects the full 256x256x1024 grid through Q/K/V/O but
returns only out[px, py] -- a single 1024-vector.  That vector depends
on one window row: 129 tokens, one query token, and the four 1024x1024
weights (by linearity, softmax shift-invariance, and sum(attn)==1):

    q      = Wq t_q + bq
    u      = Wk^T q                      (the q.bk term is constant in k
                                          -> dropped: softmax invariant)
    scores = tokens @ u
    attn   = softmax(scores/32)
    t_avg  = attn @ tokens
    out_c  = Wo_c (Wv t_avg + bv) + bo_c

v3: zero collectives (measured 25-55us each on this mesh); every core
redundantly runs the chain above and computes only its 128-row slice of
the output projection; host concatenates.  fp16 operands, fp32 PSUM.

v4: the v3 trace showed the 6.8 MiB of per-core DMA stretched over
32.5us (~218 GB/s) because (a) every load went through the single
qSPDynamicHW ring, (b) the EC-chunk loops made 30 DMA instructions that
round-gated on the 8 DMAHW semaphore lanes (each DMA waited for the
previous DMA on its lane to fully complete before issuing), and (c) the
final [128,1] out store decomposed into 128 4-byte descriptors whose
completion acks trickled in for ~7us.  v4: host-packs every operand
into a contiguous [128, bytes] block so each tensor is ONE DMA
instruction with 2 KiB descriptors, splits the big loads across both
HWDGE rings (sync + scalar) with the small ones on the gpsimd SWDGE
queue, and emits the output as a [1,128] row (stationary/moving swap in
the final matmul, bias folded in as a K=1 matmul) so the store is a
single 512-byte descriptor.
"""

import os
import sys

os.environ.setdefault("JAX_PLATFORMS", "axon,cpu")

for _p in ("/opt/trn_rl_repo", "/root/.axon_site/_ro/trn_rl_repo"):
    if os.path.isdir(_p) and _p not in sys.path:
        sys.path.append(_p)

import numpy as np

import concourse.bass as bass
import concourse.mybir as mybir
import concourse.tile as tile
from concourse import bacc
from concourse.bass_utils import run_bass_kernel_spmd

N_CORES = 8
E = 1024
EC = E // 128
WIN = 64
H = W = 256
SCALE = 1.0 / 32.0
F32 = mybir.dt.float32
F16 = mybir.dt.float16

# descriptor size knob: elements per DMA descriptor for the big loads
DESC_ELEMS = 1024  # 2 KiB fp16 descriptors

_BUILD_CACHE: dict = {}

# Lighter Tile finale: the stock _drain_and_barrier emits drain + full
# EVSEM barrier + sem clears + second barrier (~10-16us measured on this
# part).  With no collectives and per-core-independent work we keep the
# drain (output DMA completion) and sem clears behind a sem-only
# barrier, dropping the heavy drain-barrier sandwich.
from concourse.vector_clock import ScopedClock as _ScopedClock


def _light_drain_and_barrier(self, tick_clock, wait_clock):
    drain_inst = self.nc.sync.drain()
    wait_clock.add_sem_waits(
        drain_inst.ins, _ScopedClock({None: tick_clock.global_clock})
    )
    self.nc.all_engine_barrier(sem_only=True)
    popped = self.nc._tile_sem_poison_stack.pop()
    assert popped is self._sem_poison
    self.nc.clear_and_free_semaphores(list(self.sems.allocated().values()))
    self.nc.all_engine_barrier(sem_only=True)


tile.TileContext._drain_and_barrier = _light_drain_and_barrier


def _build(L: int, qidx: int):
    KA = min(128, L)          # k-chunk A: tokens [0:KA]
    ECH = EC // 2             # wv half, in chunks

    nc = bacc.Bacc(None, target_bir_lowering=False, debug=False)

    tokT_d = nc.dram_tensor("tokT", [128, EC * L], F16, kind="ExternalInput")
    tokN_d = nc.dram_tensor("tokN", [KA, EC * 128], F16, kind="ExternalInput")
    wq_d = nc.dram_tensor("wq", [128, EC * E], F16, kind="ExternalInput")
    wk_d = nc.dram_tensor("wk", [128, EC * E], F16, kind="ExternalInput")
    wvA_d = nc.dram_tensor("wvA", [128, ECH * E], F16, kind="ExternalInput")
    wvB_d = nc.dram_tensor("wvB", [128, ECH * E], F16, kind="ExternalInput")
    wo_d = nc.dram_tensor("wo", [128, EC * 128], F16, kind="ExternalInput")
    bias_d = nc.dram_tensor("biases", [128, 2 * EC], F32, kind="ExternalInput")
    bo_d = nc.dram_tensor("bo", [1, 128], F16, kind="ExternalInput")
    if L > KA:
        tokt_d = nc.dram_tensor("tokTail", [L - KA, EC * 128], F16,
                                kind="ExternalInput")
    out_d = nc.dram_tensor("out", [1, 128], F32, kind="ExternalOutput")

    FH = [slice(0, 512), slice(512, 1024)]

    with tile.TileContext(nc) as tc:
        with (
            tc.tile_pool(name="consts", bufs=1) as consts,
            tc.tile_pool(name="sbw", bufs=1) as sbw,
            tc.tile_pool(name="psS", bufs=2, space="PSUM") as psS,
        ):
            # ---- loads: one DMA instruction per tensor, split across the
            # two HWDGE rings (sync, scalar) + gpsimd SWDGE for the smalls.
            wq_sb = consts.tile([128, EC, E], F16)
            wv_sb = consts.tile([128, EC, E], F16)
            tok_sb = consts.tile([128, EC, L], F16)
            wk_sb = consts.tile([128, EC, E], F16)
            tokN_sb = consts.tile([KA, EC, 128], F16)
            wo_sb = consts.tile([128, EC, 128], F16)
            bias_sb = consts.tile([128, 2 * EC], F32)
            bo_sb = consts.tile([1, 128], F16)

            # sync ring: wq then wv (first + last weight of the chain)
            nc.sync.dma_start(out=wq_sb, in_=wq_d[:, :],
                              max_dma_last_dim=DESC_ELEMS)
            nc.sync.dma_start(out=wv_sb[:, 0:ECH, :], in_=wvA_d[:, :],
                              max_dma_last_dim=DESC_ELEMS)
            # scalar ring: tokens, wk, wv tail, wo
            nc.scalar.dma_start(out=tok_sb, in_=tokT_d[:, :],
                                max_dma_last_dim=DESC_ELEMS)
            nc.scalar.dma_start(out=wk_sb, in_=wk_d[:, :],
                                max_dma_last_dim=DESC_ELEMS)
            nc.scalar.dma_start(out=tokN_sb, in_=tokN_d[:, :],
                                max_dma_last_dim=DESC_ELEMS)
            nc.scalar.dma_start(out=wv_sb[:, ECH:EC, :], in_=wvB_d[:, :],
                                max_dma_last_dim=DESC_ELEMS)
            nc.scalar.dma_start(out=wo_sb, in_=wo_d[:, :],
                                max_dma_last_dim=DESC_ELEMS)
            # gpsimd SWDGE: tiny operands
            nc.gpsimd.dma_start(out=bias_sb, in_=bias_d[:, :])
            nc.gpsimd.dma_start(out=bo_sb, in_=bo_d[:, :])
            if L > KA:
                tokt_sb = consts.tile([L - KA, EC, 128], F16)
                nc.gpsimd.dma_start(out=tokt_sb, in_=tokt_d[:, :])

            ones16 = consts.tile([1, 128], F16)
            nc.vector.memset(ones16, 1.0)
            warm16 = consts.tile([128, 128], F16)
            nc.vector.memset(warm16, 0.0)

            # PE-HAM warmup: sustained dummy matmuls while weights stream in,
            # so the real chain runs at the unthrottled clock.
            wu_ps = psS.tile([128, 1], F32, tag="wu", bufs=1)
            for w in range(100):
                nc.tensor.matmul(wu_ps, warm16, warm16[:, 0:1],
                                 start=(w == 0), stop=(w == 99))

            # ---- q columns: q[fc] = sum_ec WqT[ec,fc]^T @ t_q (+bq) ----
            # weights stationary ([128,128] fp16 -> fast weight load)
            q_ps = psS.tile([128, EC], F32, tag="qc", bufs=1)
            for fc in range(EC):
                fsl = slice(128 * fc, 128 * (fc + 1))
                for c in range(EC):
                    nc.tensor.matmul(
                        q_ps[:, fc:fc + 1], wq_sb[:, c, fsl],
                        tok_sb[:, c, qidx:qidx + 1],
                        start=(c == 0), stop=(c == EC - 1),
                    )
            q_cols = sbw.tile([128, EC], F16)
            nc.vector.tensor_add(q_cols, q_ps, bias_sb[:, 0:EC])

            # ---- u columns: u[ec] = sum_fc WkN[fc,ec]^T @ q_col[fc] ----
            u_ps = psS.tile([128, EC], F32, tag="uc", bufs=1)
            for ec in range(EC):
                esl = slice(128 * ec, 128 * (ec + 1))
                for c in range(EC):
                    nc.tensor.matmul(
                        u_ps[:, ec:ec + 1], wk_sb[:, c, esl], q_cols[:, c:c + 1],
                        start=(c == 0), stop=(c == EC - 1),
                    )
            u_cols = sbw.tile([128, EC], F16)
            # fold the 1/sqrt(E) score scale into u
            nc.vector.tensor_scalar_mul(u_cols, u_ps, SCALE)

            # ---- scores = u^T @ tokens -> [1, L] directly in row form ----
            s_ps = psS.tile([1, L], F32, tag="sacc", bufs=1)
            for c in range(EC):
                nc.tensor.matmul(s_ps, u_cols[:, c:c + 1], tok_sb[:, c, :],
                                 start=(c == 0), stop=(c == EC - 1))

            wu2_ps = psS.tile([128, 1], F32, tag="wu", bufs=1, name="wu2_ps")
            for w in range(40):
                nc.tensor.matmul(wu2_ps, warm16, warm16[:, 0:1],
                                 start=(w == 0), stop=(w == 39))

            # ---- softmax (scores pre-scaled; |s| <= ~10 so no max-sub
            # needed for fp32 exp -- same result as the reference's
            # max-subtracted softmax) ----
            ex_row = sbw.tile([1, L], F32)
            sm = sbw.tile([1, 1], F32)
            nc.scalar.activation(ex_row, s_ps, mybir.ActivationFunctionType.Exp,
                                 bias=0.0, scale=1.0, accum_out=sm)
            rs = sbw.tile([1, 1], F32)
            nc.vector.reciprocal(rs, sm)
            at16 = sbw.tile([1, L], F16)
            nc.vector.tensor_scalar_mul(at16, ex_row, rs)

            # ---- t_avg = attn @ tokens on PE (tokens in [k, e] layout) ----
            atc_ps = psS.tile([128, 1], F16, tag="s")
            nc.tensor.transpose(atc_ps, at16[0:1, 0:KA], ones16[0:1, 0:1])
            at_colA = sbw.tile([KA, 1], F16)
            nc.vector.tensor_copy(at_colA, atc_ps)
            if L > KA:
                at_tail = sbw.tile([L - KA, 1], F16)
                nc.vector.tensor_copy(at_tail, at16[0:1, KA:L])
            tv_ps = psS.tile([128, EC], F32, tag="tv", bufs=1)
            for c in range(EC):
                nc.tensor.matmul(
                    tv_ps[:, c:c + 1], tokN_sb[:, c, :], at_colA,
                    start=True, stop=(L <= KA),
                )
                if L > KA:
                    nc.tensor.matmul(
                        tv_ps[:, c:c + 1], tokt_sb[0:1, c, :], at_tail,
                        start=False, stop=True,
                    )
            tv_cols = sbw.tile([128, EC], F16)
            nc.vector.tensor_copy(tv_cols, tv_ps)

            # ---- ctx columns: ctx[fc] = sum_ec WvT[ec,fc]^T @ t_avg[ec] + bv ----
            c_ps = psS.tile([128, EC], F32, tag="cc", bufs=1)
            for fc in range(EC):
                fsl = slice(128 * fc, 128 * (fc + 1))
                for c in range(EC):
                    nc.tensor.matmul(
                        c_ps[:, fc:fc + 1], wv_sb[:, c, fsl], tv_cols[:, c:c + 1],
                        start=(c == 0), stop=(c == EC - 1),
                    )
            ctx_cols = sbw.tile([128, EC], F16)
            nc.vector.tensor_add(ctx_cols, c_ps, bias_sb[:, EC:2 * EC])

            # ---- out row = (Wo_c ctx)^T + bo_c: ctx stationary, wo moving,
            # bias folded in as a K=1 matmul -> single-descriptor store ----
            o_ps = psS.tile([1, 128], F32, tag="s")
            for c in range(EC):
                nc.tensor.matmul(
                    o_ps, ctx_cols[:, c:c + 1], wo_sb[:, c, :],
                    start=(c == 0), stop=False,
                )
            nc.tensor.matmul(o_ps, ones16[0:1, 0:1], bo_sb[0:1, :],
                             start=False, stop=True)
            o_sb = sbw.tile([1, 128], F32)
            nc.vector.tensor_copy(o_sb, o_ps)
            nc.sync.dma_start(out=out_d[:, :], in_=o_sb)

    nc.finalize()
    return nc


def _get_nc(L: int, qidx: int):
    key = (L, qidx)
    if key not in _BUILD_CACHE:
        _BUILD_CACHE[key] = _build(L, qidx)
    return _BUILD_CACHE[key]


def _chunk_pack(a: np.ndarray) -> np.ndarray:
    """[EC*128, X] -> [128, EC*X] with [p, c*X+x] = a[c*128+p, x]."""
    n, x = a.shape
    ec = n // 128
    return np.ascontiguousarray(
        a.reshape(ec, 128, x).transpose(1, 0, 2).reshape(128, ec * x)
    )


def _prep_in_maps(matrix, Wq, bq, Wk, bk, Wv, bv, Wo, bo, px, py):
    px = int(px)
    py = int(py)
    rows = np.arange(H)[px - WIN:px + WIN + 1]
    cols = np.arange(W)[py - WIN:py + WIN + 1]
    L = len(cols)
    gr = rows[px]
    qidx = py

    tokens = np.asarray(matrix[gr][cols], dtype=np.float32)        # [L, E]
    tok16 = tokens.astype(np.float16)
    tokT_p = _chunk_pack(np.ascontiguousarray(tok16.T))            # [128, EC*L]
    KA = min(128, L)
    tokN_p = np.ascontiguousarray(tok16[0:KA])                     # [KA, E]
    wq_p = _chunk_pack(np.ascontiguousarray(
        np.asarray(Wq, np.float32).T).astype(np.float16))
    wk_p = _chunk_pack(np.asarray(Wk, np.float32).astype(np.float16))
    wv_p = _chunk_pack(np.ascontiguousarray(
        np.asarray(Wv, np.float32).T).astype(np.float16))
    ECH = EC // 2

    bq_c = np.asarray(bq, np.float32).reshape(EC, 128).T           # [128, EC]
    bv_c = np.asarray(bv, np.float32).reshape(EC, 128).T
    bias_p = np.ascontiguousarray(np.concatenate([bq_c, bv_c], axis=1))

    in_maps = []
    for c in range(N_CORES):
        fc = slice(128 * c, 128 * (c + 1))
        wo_p = _chunk_pack(np.ascontiguousarray(
            np.asarray(Wo, np.float32)[fc].T).astype(np.float16))  # [128, EC*128]
        m = {
            "tokT": tokT_p,
            "tokN": tokN_p,
            "wq": wq_p,
            "wk": wk_p,
            "wvA": np.ascontiguousarray(wv_p[:, : ECH * E]),
            "wvB": np.ascontiguousarray(wv_p[:, ECH * E:]),
            "wo": wo_p,
            "biases": bias_p,
            "bo": np.asarray(bo, np.float32)[fc].astype(np.float16)[None, :],
        }
        if L > KA:
            m["tokTail"] = np.ascontiguousarray(tok16[KA:L])
        in_maps.append(m)
    return in_maps, L, qidx


def kernel(matrix, Wq, bq, Wk, bk, Wv, bv, Wo, bo, px, py, _trace=False, **_kw):
    in_maps, L, qidx = _prep_in_maps(
        matrix, Wq, bq, Wk, bk, Wv, bv, Wo, bo, px, py
    )
    nc = _get_nc(L, qidx)
    res = run_bass_kernel_spmd(
        nc, in_maps, core_ids=list(range(N_CORES)), trace=_trace
    )
    out = np.concatenate([res.results[c]["out"][0] for c in range(N_CORES)])
    if _trace:
        return out.astype(np.float32), res
    return out.astype(np.float32)
